# revision 5
# baseline (speedup 1.0000x reference)
"""Trainium2 Bass kernel for batched weighted scatter-add (AttentionCopy).

Computes out[b, o, v] = sum_i attn[b, o, i] * (ids[b, i] == v)
for ids [16, 512] int32 in [0, 50000), attn [16, 32, 512] f32,
out [16, 32, 50000] f32.

Strategy: pure data parallel over batch (2 batches per core on 8 cores).
The output is 99% zeros (<=512 of 50000 columns are non-zero per batch), so
instead of dense one-hot matmuls (PE-bound at ~50us), the kernel:

  1. Zero-fills the whole per-core output (12.8 MB) with large coalesced
     DMAs from an SBUF zeros tile -- this runs at the HBM write roofline
     (~32-35us) and is the unavoidable cost of materializing the output.
  2. Resolves duplicate ids via the selection-matrix trick: C[j,i] =
     (ids_j == ids_i) built with 4 DVE compares, then ST = C @ attnT on the
     PE (tiny [512x512]@[512x32] matmul) so every slot i holds the full
     collision sum for its column.  Duplicate slots then hold identical
     rows, making duplicate scatter writes benign (plain overwrite).
  3. Scatters the 512 non-zero columns per batch with ONE indirect DMA
     (SWDGE) per batch: each index writes a contiguous 32-float row of the
     v-major [50000, 32] device layout.
All compute overlaps the zero-fill; scatter is ordered after it via an
SBUF WAR/RAW dependency chain (write to the zeros tile waits for the
zero-fill reads; the scatter source read waits on that write).

The device output is v-major [BPC*50000, 32]; the host unshard step
transposes each batch to the required [32, 50000] row-major layout.
"""

import sys

sys.path.insert(0, "/opt/trn_rl_repo")

import numpy as np

NCORES = 8
B, O, I = 16, 32, 512
SIZE = 50000
BPC = B // NCORES  # batches per core
NCHUNK = I // 128  # 4 id chunks of 128
ZCOLS = 1250  # zeros tile free dim: [128, 1250] f32 = 640 KB per DMA
NZDMA = SIZE * O // (128 * ZCOLS)  # 10 zero-fill DMAs per batch

_cache = {}


def _build():
    import concourse.bacc as bacc
    import concourse.bass as bass
    import concourse.mybir as mybir
    import concourse.tile as tile

    f32 = mybir.dt.float32
    bf16 = mybir.dt.bfloat16
    i32 = mybir.dt.int32
    Alu = mybir.AluOpType

    nc = bacc.Bacc("TRN2", target_bir_lowering=False, debug=False, num_devices=NCORES)

    # ids broadcast along free dim, replicated on all 128 partitions:
    # idsb[p, b*512 + i] = ids[b, i]
    idsb_d = nc.dram_tensor("idsb", [128, BPC * I], f32, kind="ExternalInput").ap()
    # per-partition ids, chunked: idspp[p, b*4 + c] = ids[b, c*128 + p]
    idspp_d = nc.dram_tensor("idspp", [128, BPC * NCHUNK], f32, kind="ExternalInput").ap()
    # scatter row indices: idx[p, b*4 + c] = b*50000 + ids[b, c*128 + p]
    idx_d = nc.dram_tensor("idx", [128, BPC * NCHUNK], i32, kind="ExternalInput").ap()
    # attn transposed: [BPC, I, O]
    attn_d = nc.dram_tensor("attn", [BPC, I, O], f32, kind="ExternalInput").ap()
    # v-major output: row r = b*50000 + v holds out[b, :, v]
    out_d = nc.dram_tensor("out", [BPC * SIZE, O], f32, kind="ExternalOutput").ap()
    out_flat = out_d.rearrange("r o -> (r o)")

    with tile.TileContext(nc) as tc:
        with (
            tc.tile_pool(name="zeros", bufs=1) as zp,
            tc.tile_pool(name="inp", bufs=1) as inp,
            tc.tile_pool(name="work", bufs=2) as wp,
            tc.tile_pool(name="vals", bufs=1) as vp,
            tc.tile_pool(name="psst", bufs=4, space="PSUM") as psp,
        ):
            # --- zeros tiles (one per batch for independent ordering chains)
            zs = []
            for b in range(BPC):
                z = zp.tile([128, ZCOLS], f32, name=f"z{b}")
                nc.gpsimd.memset(z[:], 0)
                zs.append(z)

            # --- inputs
            idsb = inp.tile([128, BPC * I], f32)
            nc.sync.dma_start(out=idsb[:], in_=idsb_d[:])
            idspp = inp.tile([128, BPC * NCHUNK], f32)
            nc.sync.dma_start(out=idspp[:], in_=idspp_d[:])
            idx = inp.tile([128, BPC * NCHUNK], i32)
            nc.sync.dma_start(out=idx[:], in_=idx_d[:])
            at_f = inp.tile([128, BPC * NCHUNK * O], f32)
            for b in range(BPC):
                for c in range(NCHUNK):
                    nc.scalar.dma_start(
                        out=at_f[:, (b * NCHUNK + c) * O : (b * NCHUNK + c + 1) * O],
                        in_=attn_d[b][c * 128 : (c + 1) * 128, :],
                    )
            atb = inp.tile([128, BPC * NCHUNK * O], bf16)
            nc.vector.tensor_copy(out=atb[:], in_=at_f[:])

            for b in range(BPC):
                # --- zero-fill this batch's 1.6M-element output region
                for k in range(NZDMA):
                    base = b * SIZE * O + k * 128 * ZCOLS
                    eng = (nc.sync, nc.scalar)[k % 2]
                    eng.dma_start(
                        out=out_flat[base : base + 128 * ZCOLS].rearrange(
                            "(p f) -> p f", f=ZCOLS
                        ),
                        in_=zs[b][:],
                    )

                # --- collision matrix C[j, i] = (ids_j == ids_i), bf16 0/1
                cmat = wp.tile([128, NCHUNK * I], bf16, tag="cmat")
                for cj in range(NCHUNK):
                    nc.vector.tensor_scalar(
                        out=cmat[:, cj * I : (cj + 1) * I],
                        in0=idsb[:, b * I : (b + 1) * I],
                        scalar1=idspp[:, b * NCHUNK + cj : b * NCHUNK + cj + 1],
                        scalar2=None,
                        op0=Alu.is_equal,
                    )

                # --- ST[i, o] = sum_j C[j, i] * attnT[j, o]  (collision sums)
                vals = vp.tile([128, NCHUNK * O], f32, name=f"v{b}")
                for ci in range(NCHUNK):
                    pst = psp.tile([128, O], f32, tag="st")
                    for cj in range(NCHUNK):
                        nc.tensor.matmul(
                            out=pst[:],
                            lhsT=cmat[:, cj * I + ci * 128 : cj * I + ci * 128 + 128],
                            rhs=atb[:, (b * NCHUNK + cj) * O : (b * NCHUNK + cj + 1) * O],
                            start=(cj == 0),
                            stop=(cj == NCHUNK - 1),
                        )
                    nc.scalar.copy(out=vals[:, ci * O : (ci + 1) * O], in_=pst[:])

                # --- ordering: scatter must land after this batch's zero-fill.
                # d1 (write zs[b]) waits on the zero-fill DMA reads (WAR);
                # d2 (write all of vals, reading zs[b]) orders after d1 (RAW);
                # the scatters (read vals) order after d2 (RAW).
                nc.gpsimd.memset(zs[b][0:1, 0:1], 0)
                nc.vector.scalar_tensor_tensor(
                    out=vals[:],
                    in0=vals[:],
                    scalar=1.0,
                    in1=zs[b][:, 0 : NCHUNK * O],
                    op0=Alu.mult,
                    op1=Alu.add,
                )

                # --- indirect scatters: one index per partition, each writing
                # a contiguous 32-float row of the v-major output
                for c in range(NCHUNK):
                    nc.gpsimd.indirect_dma_start(
                        out=out_d[:],
                        out_offset=bass.IndirectOffsetOnAxis(
                            ap=idx[:, b * NCHUNK + c : b * NCHUNK + c + 1], axis=0
                        ),
                        in_=vals[:, c * O : (c + 1) * O],
                        in_offset=None,
                    )

    nc.compile()
    return nc


def _in_maps(ids, attn):
    ids = ids.astype(np.int64)
    ids_f = ids.astype(np.float32)  # exact for values < 2**24
    in_maps = []
    for core in range(NCORES):
        idsc = ids[core * BPC : (core + 1) * BPC]  # [BPC, I]
        idsfc = ids_f[core * BPC : (core + 1) * BPC]
        # [p, b*4 + c] = ids[b, c*128 + p]
        pp = idsfc.reshape(BPC, NCHUNK, 128).transpose(2, 0, 1).reshape(128, BPC * NCHUNK)
        idxv = (idsc + (np.arange(BPC) * SIZE)[:, None]).astype(np.int32)
        idxt = idxv.reshape(BPC, NCHUNK, 128).transpose(2, 0, 1).reshape(128, BPC * NCHUNK)
        in_maps.append(
            {
                "idsb": np.ascontiguousarray(
                    np.broadcast_to(idsfc.reshape(1, BPC * I), (128, BPC * I))
                ),
                "idspp": np.ascontiguousarray(pp),
                "idx": np.ascontiguousarray(idxt),
                "attn": np.ascontiguousarray(
                    attn[core * BPC : (core + 1) * BPC].transpose(0, 2, 1)
                ),
            }
        )
    return in_maps


def kernel(ids, attn):
    from concourse.bass_utils import run_bass_kernel_spmd

    ids = np.ascontiguousarray(ids, dtype=np.int32)
    attn = np.ascontiguousarray(attn, dtype=np.float32)

    if "nc" not in _cache:
        _cache["nc"] = _build()
    nc = _cache["nc"]

    core_ids = list(range(NCORES))
    res = run_bass_kernel_spmd(nc, _in_maps(ids, attn), core_ids)
    # per-core [BPC*SIZE, 32] v-major -> [BPC, 32, SIZE] -> concat over cores
    out = np.concatenate(
        [
            res.results[c]["out"].reshape(BPC, SIZE, O).transpose(0, 2, 1)
            for c in core_ids
        ],
        axis=0,
    )
    return np.ascontiguousarray(out)
